# revision 60
# baseline (speedup 1.0000x reference)
"""AsymmetricSVD segment-reduce kernel for 8 TRN2 NeuronCores.

Strategy (data-parallel over segments, fp8 + DoubleRow):
  - Core m owns segments [512m, 512(m+1)) and their contiguous implicit
    entries (segment_ids is sorted).
  - Host precomputes per-entry scalar a_e = r_e - MU - bu[user[seg_e]] and a
    fused fp8 table XY = 128*[X | Y - bi*X] (so w*X + Y == a*X + Y'; the
    2^7 scale keeps fp8e4 out of subnormals and is folded back in Qn2).
  - Entries are bucketed by item range (4 buckets of 25000 rows so gather
    indices fit int16) and, within a bucket, grouped by 64-segment
    superblock.  Each (bucket, superblock) run is padded to a multiple of
    256 entries (cross-core max capacity, so the compiled graph is uniform
    across cores) so every 256-entry PAIR of gather groups lies inside one
    superblock -> one PSUM region (bank sb, rows 0:64 -- DoubleRow requires
    PSUM base partition 0).
  - Device gathers 256B fp8 rows per entry via gpsimd.dma_gather (SWDGE),
    4 queues, calls of 1280 descriptors (under the ~2048-desc ring
    capacity; bigger calls block the single Q7 inside one call while the
    other queues run dry).  The SWDGE random-gather wall is ~2ns per
    256B descriptor aggregate; descriptor count, not bytes, dominates.
  - The one-hot/coefficient lhsT tiles are PRE-BUILT ON HOST in fp8,
    group-major ([128, groups, 2, 64]: Sp, S per 128-entry group; in-run
    duplicate items merged into multi-hot columns) and streamed via HWDGE -
    no on-device one-hot construction at all.
  - Adjacent group pairs run as fp8 DoubleRow matmuls (256 entries each,
    0.5 cyc/row, k-tiles = the two groups); odd trailing groups of a run
    use plain fp8 matmuls:
        PSUM[sb][0:64, 0:128]   += sum_e a_e * X_e      (lhsT = Sp)
        PSUM[sb][0:64, 128:256] += sum_e (Y')_e         (lhsT = S)
  - Chunks run superblock-major across buckets, so each superblock's
    accumulation closes and its epilogue runs while later superblocks are
    still gathering.
  - Epilogue: rui[seg] = bui[seg] + reduce_add(PSUM[seg, 0:256] * Qn2[seg])
    with Qn2 = [Qn | Qn], Qn = Q[item]*norm/128 precomputed on host.

Measured: ~275us HW exec on 8 cores, rel err ~1.7e-5.
"""

import numpy as np
import ml_dtypes

MU = 3.5
B = 4096
F = 128
NUM_ITEMS = 100000
N_CORES = 8
SEGS_PER_CORE = B // N_CORES            # 512
N_BUCKETS = 4
BUCKET_ROWS = (NUM_ITEMS + N_BUCKETS - 1) // N_BUCKETS   # 25000 < 32768 (int16)
SB = 64                                  # segments per superblock
NSB = SEGS_PER_CORE // SB                # 8 superblocks per core
PAIR = 256                               # entries per DoubleRow matmul pair
# Gather calls must stay under the ~2048-descriptor SWDGE ring capacity:
# larger calls block the single Q7 inside one call while the other queues
# run dry (measured 20us stalls per oversized call).
CHUNK = 1280                             # entries per dma_gather call (5 pairs)
                                         # fits the ~2048-desc ring with slack
N_QUEUES = 4                             # SWDGE gather queues (ucode max)
FP8 = ml_dtypes.float8_e4m3
XSCALE = 128.0                           # fp8 range scale for X/Y' rows


def _host_prep(bu, bi, Q, X, Y, user, item, imp_items, imp_ratings, segment_ids):
    """All index/scalar preprocessing. Returns per-core device arrays and
    uniform cross-core metadata for codegen."""
    a_full = imp_ratings.astype(np.float32) - MU - bu[user[segment_ids], 0]
    Yp = Y - bi * X                                    # [NUM_ITEMS, F]
    XYs = np.concatenate([X * XSCALE, Yp * XSCALE], axis=1).astype(FP8)

    counts = np.bincount(segment_ids, minlength=B).astype(np.float32)
    norm = np.where(counts > 0, counts, 1.0) ** -0.5
    bui = (MU + bu[user, 0] + bi[item, 0]).astype(np.float32)          # [B]
    Qh = (Q[item] * (norm / XSCALE)[:, None]).astype(np.float32)       # [B, F]
    Qn2 = np.concatenate([Qh, Qh], axis=1)                             # [B, 256]

    # --- shard entries by segment block; group by (bucket, superblock).
    # Keep segment order within runs: ascending-item (HBM-sorted) descriptor
    # order measured SLOWER (channel serialization), so don't sort by item.
    bounds = np.searchsorted(segment_ids, np.arange(0, B + 1, SEGS_PER_CORE))
    percore = []
    cnt = np.zeros((N_CORES, N_BUCKETS, NSB), np.int64)
    for m in range(N_CORES):
        lo, hi = bounds[m], bounds[m + 1]
        it = imp_items[lo:hi]
        sl = (segment_ids[lo:hi] - m * SEGS_PER_CORE).astype(np.int64)
        av = a_full[lo:hi]
        bk = it // BUCKET_ROWS
        key = bk * NSB + sl // SB
        order = np.argsort(key, kind="stable")
        it, sl, av, key = it[order], sl[order], av[order], key[order]
        ne = it.shape[0]
        # merge duplicate items within a run (multi-hot lhsT column, one
        # gather descriptor), keeping first-occurrence order so descriptor
        # addresses stay random (item-sorted order measured slower).
        pos = np.arange(ne)
        o2 = np.lexsort((pos, it, key))
        k2, i2, p2 = key[o2], it[o2], pos[o2]
        new = np.ones(ne, bool)
        new[1:] = (k2[1:] != k2[:-1]) | (i2[1:] != i2[:-1])
        gid = np.cumsum(new) - 1
        first_of = np.empty(ne, np.int64)
        first_of[o2] = p2[new][gid]          # first occurrence position
        rep = pos == first_of                # representative entries
        cnt[m] = np.bincount(key[rep], minlength=N_BUCKETS * NSB).reshape(
            N_BUCKETS, NSB)
        percore.append((it, sl, av, key, rep, first_of))

    # uniform per-(bucket, superblock) capacity: cross-core max, rounded to
    # 128 (gather-group granularity; an odd trailing group uses plain
    # matmuls instead of a DoubleRow pair)
    cap = ((cnt.max(axis=0) + 127) // 128) * 128               # [4, 8]
    offs_flat = np.concatenate([[0], np.cumsum(cap.ravel())])
    E_pad = int(offs_flat[-1])
    Gg = E_pad // 128                                          # 128-groups

    # group -> psum bank metadata (uniform across cores). DoubleRow matmuls
    # require PSUM base partition 0, so superblock sb owns rows 0:64, cols
    # 0:256 of bank sb.
    group_meta = []
    for b in range(N_BUCKETS):
        for sb in range(NSB):
            group_meta.extend([sb] * (int(cap[b, sb]) // 128))
    assert len(group_meta) == Gg

    # chunk list in SUPERBLOCK-MAJOR order across buckets so each
    # superblock's accumulation closes (and its epilogue runs) while later
    # superblocks are still gathering.  chunks[sb] = [(start, n, bucket)];
    # every chunk has an even group count except the last chunk of an
    # odd-group run (its final group is emitted as plain matmuls).
    bucket_bounds = []
    for b in range(N_BUCKETS):
        bucket_bounds.append((int(offs_flat[b * NSB]), int(cap[b].sum())))
    chunks = []
    for sb in range(NSB):
        sb_chunks = []
        for b in range(N_BUCKETS):
            s = int(offs_flat[b * NSB + sb])
            nrun = int(cap[b, sb])
            ng = nrun // 128
            ncalls = max(1, (nrun + CHUNK - 1) // CHUNK)
            base_p, extra = divmod(ng // 2, ncalls)
            pieces = [2 * (base_p + (1 if c < extra else 0))
                      for c in range(ncalls)]
            pieces[-1] += ng % 2
            for g in pieces:
                if g == 0:
                    continue
                n = g * 128
                sb_chunks.append((s, n, b))
                s += n
        chunks.append(sb_chunks)

    meta = dict(E_pad=E_pad, G2=Gg, chunks=chunks, group_meta=group_meta,
                bucket_bounds=bucket_bounds)

    # --- per-core device arrays ---
    def wrap16(x):   # entry e -> [e%16, e//16], replicated to 128 partitions
        w = x.reshape(-1, 16).T
        return np.ascontiguousarray(np.tile(w, (8, 1)))

    in_maps = []
    for m in range(N_CORES):
        it, sl, av, key, rep, first_of = percore[m]
        ne = it.shape[0]
        # slots go to representatives (in run-stable order); every entry
        # maps to its representative's slot
        gstart = np.concatenate([[0], np.cumsum(cnt[m].ravel())])
        nrep = int(rep.sum())
        krep = key[rep]
        rank = np.arange(nrep) - gstart[krep]
        slot_rep = offs_flat[krep] + rank
        srep = np.zeros(ne, np.int64)
        srep[rep] = slot_rep
        slot = srep[first_of]

        lidx = np.zeros(E_pad, np.int16)
        lidx[slot_rep] = (it[rep] - (it[rep] // BUCKET_ROWS)
                          * BUCKET_ROWS).astype(np.int16)

        # lhsT coefficients, group-major: [128, Gg, 2, 64] = (Sp, S) per
        # 128-group; accumulated so merged duplicates get multi-hot columns
        g = slot // 128
        part = slot % 128
        col = sl - SB * (key % NSB)
        LTf = np.zeros((128, Gg, 2, SB), np.float32)
        np.add.at(LTf, (part, g, 0, col), av)
        np.add.at(LTf, (part, g, 1, col), 1.0)

        in_maps.append({
            "xy": XYs,
            "idx16": wrap16(lidx),
            "lt": LTf.astype(FP8),
            "qn2": np.ascontiguousarray(Qn2[m * SEGS_PER_CORE:(m + 1) * SEGS_PER_CORE]),
            "bui": np.ascontiguousarray(bui[m * SEGS_PER_CORE:(m + 1) * SEGS_PER_CORE]),
        })
    return in_maps, meta


def _build_graph(meta):
    from concourse import bacc, mybir
    from concourse.tile import TileContext

    E_pad, Gg = meta["E_pad"], meta["G2"]
    chunks, group_meta = meta["chunks"], meta["group_meta"]
    bucket_bounds = meta["bucket_bounds"]

    nc = bacc.Bacc("TRN2", target_bir_lowering=False, debug=False,
                   num_devices=N_CORES, num_swdge_queues=N_QUEUES)
    fp8, f32, i16 = mybir.dt.float8e4, mybir.dt.float32, mybir.dt.int16
    DR = mybir.MatmulPerfMode.DoubleRow

    xy_d = nc.declare_dram_parameter("xy", [NUM_ITEMS, 256], fp8, isOutput=False)
    idx_d = nc.declare_dram_parameter("idx16", [128, E_pad // 16], i16, isOutput=False)
    lt_d = nc.declare_dram_parameter("lt", [128, Gg, 2, SB], fp8, isOutput=False)
    qn_d = nc.declare_dram_parameter("qn2", [SEGS_PER_CORE, 256], f32, isOutput=False)
    bui_d = nc.declare_dram_parameter("bui", [SEGS_PER_CORE], f32, isOutput=False)
    out_d = nc.declare_dram_parameter("out", [SEGS_PER_CORE], f32, isOutput=True)

    # 8 PSUM bank tiles; superblock sb lives at rows 0:64, cols 0:256 of
    # bank sb.
    n_tiles = NSB  # 8

    with TileContext(nc) as tc:
        with (
            tc.tile_pool(name="const", bufs=1) as cpool,
            tc.tile_pool(name="xy", bufs=24) as xypool,
            tc.tile_pool(name="epi", bufs=2) as epool,
            tc.tile_pool(name="psum", bufs=1, space="PSUM") as ppool,
        ):
            # per-bucket idx + lhsT tiles; idx on the sync HWDGE queue
            # (gates gathers), lhsT on the scalar queue.  Bucket 0's idx is
            # split so the first gather calls aren't gated on a large load.
            idx_tiles, lt_tiles, lt_poff = [], [], []
            HEAD = 4096
            # all four HEAD loads first (they gate the first gather wave),
            # then the big rests
            for b in range(N_BUCKETS):
                boff, bn = bucket_bounds[b]
                if bn == 0:
                    idx_tiles.append(None)
                    continue
                t = cpool.tile([128, bn // 16], i16, tag=f"idx{b}")
                nc.sync.dma_start(
                    out=t[:, 0:min(HEAD, bn) // 16],
                    in_=idx_d[:, boff // 16:(boff + min(HEAD, bn)) // 16])
                idx_tiles.append(t)
            for b in range(N_BUCKETS):
                boff, bn = bucket_bounds[b]
                if bn > HEAD:
                    nc.sync.dma_start(
                        out=idx_tiles[b][:, HEAD // 16:bn // 16],
                        in_=idx_d[:, (boff + HEAD) // 16:(boff + bn) // 16])
            for b in range(N_BUCKETS):
                boff, bn = bucket_bounds[b]
                if bn == 0:
                    lt_tiles.append(None)
                    lt_poff.append(0)
                    continue
                g0b, ngb = boff // 128, bn // 128
                lt = cpool.tile([128, ngb, 2, SB], fp8, tag=f"lt{b}")
                nc.scalar.dma_start(out=lt[:], in_=lt_d[:, g0b:g0b + ngb, :, :])
                lt_tiles.append(lt)
                lt_poff.append(g0b)

            # epilogue inputs preloaded upfront (no dependencies)
            qn_t = cpool.tile([128, NSB, 256], f32, tag="qn")
            bui_t = cpool.tile([128, NSB], f32, tag="bui")
            for sb in range(NSB):
                nc.scalar.dma_start(
                    out=qn_t[0:SB, sb, :], in_=qn_d[SB * sb:SB * sb + SB, :])
                nc.scalar.dma_start(
                    out=bui_t[0:SB, sb:sb + 1],
                    in_=bui_d[SB * sb:SB * sb + SB])

            zeros_t = cpool.tile([128, 512], fp8, tag="zeros")
            nc.vector.memset(zeros_t[:], 0.0)

            psum_t = []
            for kbank in range(n_tiles):
                pt = ppool.tile([128, 512], f32, tag=f"bank{kbank}")
                psum_t.append(pt)
                nc.tensor.matmul(
                    out=pt[0:SB, 0:512], lhsT=zeros_t[:, 0:SB],
                    rhs=zeros_t[:, 0:512], start=True, stop=False,
                )

            ci = 0
            for sb in range(NSB):
                for (start, n, b) in chunks[sb]:
                    nG = n // 128
                    boff = bucket_bounds[b][0]
                    xyt = xypool.tile([128, nG, 256], fp8, tag="xyt")
                    nc.gpsimd.dma_gather(
                        out_ap=xyt[:],
                        in_ap=xy_d[b * BUCKET_ROWS:(b + 1) * BUCKET_ROWS, :],
                        idxs_ap=idx_tiles[b][:, (start - boff) // 16:
                                             (start - boff + n) // 16],
                        num_idxs=n,
                        num_idxs_reg=n,
                        elem_size=256,
                        single_packet=False,
                        queue_num=ci % N_QUEUES,
                    )
                    ci += 1
                    ltb = lt_tiles[b]
                    u = 0
                    while u < nG:
                        Gi = start // 128 + u
                        Gb = Gi - lt_poff[b]
                        bank = group_meta[Gi]
                        if u + 1 < nG:
                            # DoubleRow pair: k-tiles = groups u, u+1
                            for which in (0, 1):
                                c0, c1 = 128 * which, 128 * (which + 1)
                                nc.tensor.matmul(
                                    out=psum_t[bank][0:SB, c0:c1],
                                    lhsT=ltb[:, Gb:Gb + 2, which, :],
                                    rhs=xyt[:, u:u + 2, c0:c1],
                                    start=False, stop=False,
                                    perf_mode=DR,
                                )
                            u += 2
                        else:
                            # odd trailing group: plain fp8 matmuls
                            for which in (0, 1):
                                c0, c1 = 128 * which, 128 * (which + 1)
                                nc.tensor.matmul(
                                    out=psum_t[bank][0:SB, c0:c1],
                                    lhsT=ltb[:, Gb, which, :],
                                    rhs=xyt[:, u, c0:c1],
                                    start=False, stop=False,
                                )
                            u += 1

                # this superblock's bank is done: close its accumulation
                # group (zero-region = full 2KB bank) and run its epilogue
                # while later superblocks are still gathering.
                nc.tensor.matmul(
                    out=psum_t[sb][0:SB, 0:512], lhsT=zeros_t[:, 0:SB],
                    rhs=zeros_t[:, 0:512], start=False, stop=True,
                )
                s0 = SB * sb
                prod_t = epool.tile([128, 256], f32, tag="prod")
                nc.vector.tensor_tensor(
                    out=prod_t[0:SB, :],
                    in0=psum_t[sb][0:SB, 0:256],
                    in1=qn_t[0:SB, sb, :],
                    op=mybir.AluOpType.mult,
                )
                red_t = epool.tile([128, 1], f32, tag="red")
                nc.vector.tensor_reduce(
                    out=red_t[0:SB, 0:1], in_=prod_t[0:SB, :],
                    axis=mybir.AxisListType.X,
                    op=mybir.AluOpType.add,
                )
                nc.vector.tensor_add(red_t[0:SB, 0:1], red_t[0:SB, 0:1],
                                     bui_t[0:SB, sb:sb + 1])
                nc.sync.dma_start(
                    out=out_d[s0:s0 + SB], in_=red_t[0:SB, 0:1])

    nc.compile()
    return nc


def kernel(bu, bi, Q, X, Y, user, item, imp_items, imp_ratings, segment_ids,
           _sim=False):
    bu = np.asarray(bu, np.float32)
    bi = np.asarray(bi, np.float32)
    Q = np.asarray(Q, np.float32)
    X = np.asarray(X, np.float32)
    Y = np.asarray(Y, np.float32)
    user = np.asarray(user).astype(np.int64)
    item = np.asarray(item).astype(np.int64)
    imp_items = np.asarray(imp_items).astype(np.int64)
    imp_ratings = np.asarray(imp_ratings).astype(np.int64)
    segment_ids = np.asarray(segment_ids).astype(np.int64)

    in_maps, meta = _host_prep(bu, bi, Q, X, Y, user, item, imp_items,
                               imp_ratings, segment_ids)
    nc = _build_graph(meta)

    if _sim:
        from concourse import bass_interp
        sim = bass_interp.CoreSim(nc)
        sim.assign_tensors(in_maps[0])
        sim.simulate()
        out0 = np.array(sim.tensor("out"))
        return sim, out0, in_maps, meta

    from concourse.bass_utils import run_bass_kernel_spmd
    res = run_bass_kernel_spmd(nc, in_maps, core_ids=list(range(N_CORES)),
                               trace=False)
    out = np.concatenate([res.results[m]["out"] for m in range(N_CORES)])
    return out.astype(np.float32)


# revision 61
# speedup vs baseline: 1.0673x; 1.0673x over previous
"""AsymmetricSVD segment-reduce kernel for 8 TRN2 NeuronCores.

Strategy (data-parallel over segments, fp8 + DoubleRow):
  - Core m owns segments [512m, 512(m+1)) and their contiguous implicit
    entries (segment_ids is sorted).
  - Host precomputes per-entry scalar a_e = r_e - MU - bu[user[seg_e]] and a
    fused fp8 table XY = 128*[X | Y - bi*X] (so w*X + Y == a*X + Y'; the
    2^7 scale keeps fp8e4 out of subnormals and is folded back in Qn2).
  - Entries are bucketed by item range (4 buckets of 25000 rows so gather
    indices fit int16) and, within a bucket, grouped by 64-segment
    superblock.  Each (bucket, superblock) run is padded to a multiple of
    256 entries (cross-core max capacity, so the compiled graph is uniform
    across cores) so every 256-entry PAIR of gather groups lies inside one
    superblock -> one PSUM region (bank sb, rows 0:64 -- DoubleRow requires
    PSUM base partition 0).
  - Device gathers 256B fp8 rows per entry via gpsimd.dma_gather (SWDGE),
    4 queues, calls of 1280 descriptors (under the ~2048-desc ring
    capacity; bigger calls block the single Q7 inside one call while the
    other queues run dry).  The SWDGE random-gather wall is ~2ns per
    256B descriptor aggregate; descriptor count, not bytes, dominates.
  - The one-hot/coefficient lhsT tiles are PRE-BUILT ON HOST in fp8,
    group-major ([128, groups, 2, 64]: Sp, S per 128-entry group; in-run
    duplicate items merged into multi-hot columns) and streamed via HWDGE -
    no on-device one-hot construction at all.
  - Adjacent group pairs run as fp8 DoubleRow matmuls (256 entries each,
    0.5 cyc/row, k-tiles = the two groups); odd trailing groups of a run
    use plain fp8 matmuls:
        PSUM[sb][0:64, 0:128]   += sum_e a_e * X_e      (lhsT = Sp)
        PSUM[sb][0:64, 128:256] += sum_e (Y')_e         (lhsT = S)
  - Chunks run superblock-major across buckets, so each superblock's
    accumulation closes and its epilogue runs while later superblocks are
    still gathering.
  - Epilogue: rui[seg] = bui[seg] + reduce_add(PSUM[seg, 0:256] * Qn2[seg])
    with Qn2 = [Qn | Qn], Qn = Q[item]*norm/128 precomputed on host.

Measured: ~275us HW exec on 8 cores, rel err ~1.7e-5.
"""

import numpy as np
import ml_dtypes

MU = 3.5
B = 4096
F = 128
NUM_ITEMS = 100000
N_CORES = 8
SEGS_PER_CORE = B // N_CORES            # 512
N_BUCKETS = 4
BUCKET_ROWS = (NUM_ITEMS + N_BUCKETS - 1) // N_BUCKETS   # 25000 < 32768 (int16)
SB = 64                                  # segments per superblock
NSB = SEGS_PER_CORE // SB                # 8 superblocks per core
PAIR = 256                               # entries per DoubleRow matmul pair
# Gather calls must stay under the ~2048-descriptor SWDGE ring capacity:
# larger calls block the single Q7 inside one call while the other queues
# run dry (measured 20us stalls per oversized call).
CHUNK = 1664                             # entries per dma_gather call (5 pairs)
                                         # fits the ~2048-desc ring with slack
N_QUEUES = 4                             # SWDGE gather queues (ucode max)
FP8 = ml_dtypes.float8_e4m3
XSCALE = 128.0                           # fp8 range scale for X/Y' rows


def _host_prep(bu, bi, Q, X, Y, user, item, imp_items, imp_ratings, segment_ids):
    """All index/scalar preprocessing. Returns per-core device arrays and
    uniform cross-core metadata for codegen."""
    a_full = imp_ratings.astype(np.float32) - MU - bu[user[segment_ids], 0]
    Yp = Y - bi * X                                    # [NUM_ITEMS, F]
    XYs = np.concatenate([X * XSCALE, Yp * XSCALE], axis=1).astype(FP8)

    counts = np.bincount(segment_ids, minlength=B).astype(np.float32)
    norm = np.where(counts > 0, counts, 1.0) ** -0.5
    bui = (MU + bu[user, 0] + bi[item, 0]).astype(np.float32)          # [B]
    Qh = (Q[item] * (norm / XSCALE)[:, None]).astype(np.float32)       # [B, F]
    Qn2 = np.concatenate([Qh, Qh], axis=1)                             # [B, 256]

    # --- shard entries by segment block; group by (bucket, superblock).
    # Keep segment order within runs: ascending-item (HBM-sorted) descriptor
    # order measured SLOWER (channel serialization), so don't sort by item.
    bounds = np.searchsorted(segment_ids, np.arange(0, B + 1, SEGS_PER_CORE))
    percore = []
    cnt = np.zeros((N_CORES, N_BUCKETS, NSB), np.int64)
    for m in range(N_CORES):
        lo, hi = bounds[m], bounds[m + 1]
        it = imp_items[lo:hi]
        sl = (segment_ids[lo:hi] - m * SEGS_PER_CORE).astype(np.int64)
        av = a_full[lo:hi]
        bk = it // BUCKET_ROWS
        key = bk * NSB + sl // SB
        order = np.argsort(key, kind="stable")
        it, sl, av, key = it[order], sl[order], av[order], key[order]
        ne = it.shape[0]
        # merge duplicate items within a run (multi-hot lhsT column, one
        # gather descriptor), keeping first-occurrence order so descriptor
        # addresses stay random (item-sorted order measured slower).
        pos = np.arange(ne)
        o2 = np.lexsort((pos, it, key))
        k2, i2, p2 = key[o2], it[o2], pos[o2]
        new = np.ones(ne, bool)
        new[1:] = (k2[1:] != k2[:-1]) | (i2[1:] != i2[:-1])
        gid = np.cumsum(new) - 1
        first_of = np.empty(ne, np.int64)
        first_of[o2] = p2[new][gid]          # first occurrence position
        rep = pos == first_of                # representative entries
        cnt[m] = np.bincount(key[rep], minlength=N_BUCKETS * NSB).reshape(
            N_BUCKETS, NSB)
        percore.append((it, sl, av, key, rep, first_of))

    # uniform per-(bucket, superblock) capacity: cross-core max, rounded to
    # 128 (gather-group granularity; an odd trailing group uses plain
    # matmuls instead of a DoubleRow pair)
    cap = ((cnt.max(axis=0) + 127) // 128) * 128               # [4, 8]
    offs_flat = np.concatenate([[0], np.cumsum(cap.ravel())])
    E_pad = int(offs_flat[-1])
    Gg = E_pad // 128                                          # 128-groups

    # group -> psum bank metadata (uniform across cores). DoubleRow matmuls
    # require PSUM base partition 0, so superblock sb owns rows 0:64, cols
    # 0:256 of bank sb.
    group_meta = []
    for b in range(N_BUCKETS):
        for sb in range(NSB):
            group_meta.extend([sb] * (int(cap[b, sb]) // 128))
    assert len(group_meta) == Gg

    # chunk list in SUPERBLOCK-MAJOR order across buckets so each
    # superblock's accumulation closes (and its epilogue runs) while later
    # superblocks are still gathering.  chunks[sb] = [(start, n, bucket)];
    # every chunk has an even group count except the last chunk of an
    # odd-group run (its final group is emitted as plain matmuls).
    bucket_bounds = []
    for b in range(N_BUCKETS):
        bucket_bounds.append((int(offs_flat[b * NSB]), int(cap[b].sum())))
    chunks = []
    for sb in range(NSB):
        sb_chunks = []
        for b in range(N_BUCKETS):
            s = int(offs_flat[b * NSB + sb])
            nrun = int(cap[b, sb])
            ng = nrun // 128
            ncalls = max(1, (nrun + CHUNK - 1) // CHUNK)
            base_p, extra = divmod(ng // 2, ncalls)
            pieces = [2 * (base_p + (1 if c < extra else 0))
                      for c in range(ncalls)]
            pieces[-1] += ng % 2
            for g in pieces:
                if g == 0:
                    continue
                n = g * 128
                sb_chunks.append((s, n, b))
                s += n
        chunks.append(sb_chunks)

    meta = dict(E_pad=E_pad, G2=Gg, chunks=chunks, group_meta=group_meta,
                bucket_bounds=bucket_bounds)

    # --- per-core device arrays ---
    def wrap16(x):   # entry e -> [e%16, e//16], replicated to 128 partitions
        w = x.reshape(-1, 16).T
        return np.ascontiguousarray(np.tile(w, (8, 1)))

    in_maps = []
    for m in range(N_CORES):
        it, sl, av, key, rep, first_of = percore[m]
        ne = it.shape[0]
        # slots go to representatives (in run-stable order); every entry
        # maps to its representative's slot
        gstart = np.concatenate([[0], np.cumsum(cnt[m].ravel())])
        nrep = int(rep.sum())
        krep = key[rep]
        rank = np.arange(nrep) - gstart[krep]
        slot_rep = offs_flat[krep] + rank
        srep = np.zeros(ne, np.int64)
        srep[rep] = slot_rep
        slot = srep[first_of]

        lidx = np.zeros(E_pad, np.int16)
        lidx[slot_rep] = (it[rep] - (it[rep] // BUCKET_ROWS)
                          * BUCKET_ROWS).astype(np.int16)

        # lhsT coefficients, group-major: [128, Gg, 2, 64] = (Sp, S) per
        # 128-group; accumulated so merged duplicates get multi-hot columns
        g = slot // 128
        part = slot % 128
        col = sl - SB * (key % NSB)
        LTf = np.zeros((128, Gg, 2, SB), np.float32)
        np.add.at(LTf, (part, g, 0, col), av)
        np.add.at(LTf, (part, g, 1, col), 1.0)

        in_maps.append({
            "xy": XYs,
            "idx16": wrap16(lidx),
            "lt": LTf.astype(FP8),
            "qn2": np.ascontiguousarray(Qn2[m * SEGS_PER_CORE:(m + 1) * SEGS_PER_CORE]),
            "bui": np.ascontiguousarray(bui[m * SEGS_PER_CORE:(m + 1) * SEGS_PER_CORE]),
        })
    return in_maps, meta


def _build_graph(meta):
    from concourse import bacc, mybir
    from concourse.tile import TileContext

    E_pad, Gg = meta["E_pad"], meta["G2"]
    chunks, group_meta = meta["chunks"], meta["group_meta"]
    bucket_bounds = meta["bucket_bounds"]

    nc = bacc.Bacc("TRN2", target_bir_lowering=False, debug=False,
                   num_devices=N_CORES, num_swdge_queues=N_QUEUES)
    fp8, f32, i16 = mybir.dt.float8e4, mybir.dt.float32, mybir.dt.int16
    DR = mybir.MatmulPerfMode.DoubleRow

    xy_d = nc.declare_dram_parameter("xy", [NUM_ITEMS, 256], fp8, isOutput=False)
    idx_d = nc.declare_dram_parameter("idx16", [128, E_pad // 16], i16, isOutput=False)
    lt_d = nc.declare_dram_parameter("lt", [128, Gg, 2, SB], fp8, isOutput=False)
    qn_d = nc.declare_dram_parameter("qn2", [SEGS_PER_CORE, 256], f32, isOutput=False)
    bui_d = nc.declare_dram_parameter("bui", [SEGS_PER_CORE], f32, isOutput=False)
    out_d = nc.declare_dram_parameter("out", [SEGS_PER_CORE], f32, isOutput=True)

    # 8 PSUM bank tiles; superblock sb lives at rows 0:64, cols 0:256 of
    # bank sb.
    n_tiles = NSB  # 8

    with TileContext(nc) as tc:
        with (
            tc.tile_pool(name="const", bufs=1) as cpool,
            tc.tile_pool(name="xy", bufs=24) as xypool,
            tc.tile_pool(name="epi", bufs=2) as epool,
            tc.tile_pool(name="psum", bufs=1, space="PSUM") as ppool,
        ):
            # per-bucket idx + lhsT tiles; idx on the sync HWDGE queue
            # (gates gathers), lhsT on the scalar queue.  Bucket 0's idx is
            # split so the first gather calls aren't gated on a large load.
            idx_tiles, lt_tiles, lt_poff = [], [], []
            HEAD = 4096
            # all four HEAD loads first (they gate the first gather wave),
            # then the big rests
            for b in range(N_BUCKETS):
                boff, bn = bucket_bounds[b]
                if bn == 0:
                    idx_tiles.append(None)
                    continue
                t = cpool.tile([128, bn // 16], i16, tag=f"idx{b}")
                nc.sync.dma_start(
                    out=t[:, 0:min(HEAD, bn) // 16],
                    in_=idx_d[:, boff // 16:(boff + min(HEAD, bn)) // 16])
                idx_tiles.append(t)
            for b in range(N_BUCKETS):
                boff, bn = bucket_bounds[b]
                if bn > HEAD:
                    nc.sync.dma_start(
                        out=idx_tiles[b][:, HEAD // 16:bn // 16],
                        in_=idx_d[:, (boff + HEAD) // 16:(boff + bn) // 16])
            for b in range(N_BUCKETS):
                boff, bn = bucket_bounds[b]
                if bn == 0:
                    lt_tiles.append(None)
                    lt_poff.append(0)
                    continue
                g0b, ngb = boff // 128, bn // 128
                lt = cpool.tile([128, ngb, 2, SB], fp8, tag=f"lt{b}")
                nc.scalar.dma_start(out=lt[:], in_=lt_d[:, g0b:g0b + ngb, :, :])
                lt_tiles.append(lt)
                lt_poff.append(g0b)

            # epilogue inputs preloaded upfront (no dependencies)
            qn_t = cpool.tile([128, NSB, 256], f32, tag="qn")
            bui_t = cpool.tile([128, NSB], f32, tag="bui")
            for sb in range(NSB):
                nc.scalar.dma_start(
                    out=qn_t[0:SB, sb, :], in_=qn_d[SB * sb:SB * sb + SB, :])
                nc.scalar.dma_start(
                    out=bui_t[0:SB, sb:sb + 1],
                    in_=bui_d[SB * sb:SB * sb + SB])

            zeros_t = cpool.tile([128, 512], fp8, tag="zeros")
            nc.vector.memset(zeros_t[:], 0.0)

            psum_t = []
            for kbank in range(n_tiles):
                pt = ppool.tile([128, 512], f32, tag=f"bank{kbank}")
                psum_t.append(pt)
                nc.tensor.matmul(
                    out=pt[0:SB, 0:512], lhsT=zeros_t[:, 0:SB],
                    rhs=zeros_t[:, 0:512], start=True, stop=False,
                )

            ci = 0
            for sb in range(NSB):
                for (start, n, b) in chunks[sb]:
                    nG = n // 128
                    boff = bucket_bounds[b][0]
                    xyt = xypool.tile([128, nG, 256], fp8, tag="xyt")
                    nc.gpsimd.dma_gather(
                        out_ap=xyt[:],
                        in_ap=xy_d[b * BUCKET_ROWS:(b + 1) * BUCKET_ROWS, :],
                        idxs_ap=idx_tiles[b][:, (start - boff) // 16:
                                             (start - boff + n) // 16],
                        num_idxs=n,
                        num_idxs_reg=n,
                        elem_size=256,
                        single_packet=False,
                        queue_num=ci % N_QUEUES,
                    )
                    ci += 1
                    ltb = lt_tiles[b]
                    u = 0
                    while u < nG:
                        Gi = start // 128 + u
                        Gb = Gi - lt_poff[b]
                        bank = group_meta[Gi]
                        if u + 1 < nG:
                            # DoubleRow pair: k-tiles = groups u, u+1
                            for which in (0, 1):
                                c0, c1 = 128 * which, 128 * (which + 1)
                                nc.tensor.matmul(
                                    out=psum_t[bank][0:SB, c0:c1],
                                    lhsT=ltb[:, Gb:Gb + 2, which, :],
                                    rhs=xyt[:, u:u + 2, c0:c1],
                                    start=False, stop=False,
                                    perf_mode=DR,
                                )
                            u += 2
                        else:
                            # odd trailing group: plain fp8 matmuls
                            for which in (0, 1):
                                c0, c1 = 128 * which, 128 * (which + 1)
                                nc.tensor.matmul(
                                    out=psum_t[bank][0:SB, c0:c1],
                                    lhsT=ltb[:, Gb, which, :],
                                    rhs=xyt[:, u, c0:c1],
                                    start=False, stop=False,
                                )
                            u += 1

                # this superblock's bank is done: close its accumulation
                # group (zero-region = full 2KB bank) and run its epilogue
                # while later superblocks are still gathering.
                nc.tensor.matmul(
                    out=psum_t[sb][0:SB, 0:512], lhsT=zeros_t[:, 0:SB],
                    rhs=zeros_t[:, 0:512], start=False, stop=True,
                )
                s0 = SB * sb
                prod_t = epool.tile([128, 256], f32, tag="prod")
                nc.vector.tensor_tensor(
                    out=prod_t[0:SB, :],
                    in0=psum_t[sb][0:SB, 0:256],
                    in1=qn_t[0:SB, sb, :],
                    op=mybir.AluOpType.mult,
                )
                red_t = epool.tile([128, 1], f32, tag="red")
                nc.vector.tensor_reduce(
                    out=red_t[0:SB, 0:1], in_=prod_t[0:SB, :],
                    axis=mybir.AxisListType.X,
                    op=mybir.AluOpType.add,
                )
                nc.vector.tensor_add(red_t[0:SB, 0:1], red_t[0:SB, 0:1],
                                     bui_t[0:SB, sb:sb + 1])
                nc.sync.dma_start(
                    out=out_d[s0:s0 + SB], in_=red_t[0:SB, 0:1])

    nc.compile()
    return nc


def kernel(bu, bi, Q, X, Y, user, item, imp_items, imp_ratings, segment_ids,
           _sim=False):
    bu = np.asarray(bu, np.float32)
    bi = np.asarray(bi, np.float32)
    Q = np.asarray(Q, np.float32)
    X = np.asarray(X, np.float32)
    Y = np.asarray(Y, np.float32)
    user = np.asarray(user).astype(np.int64)
    item = np.asarray(item).astype(np.int64)
    imp_items = np.asarray(imp_items).astype(np.int64)
    imp_ratings = np.asarray(imp_ratings).astype(np.int64)
    segment_ids = np.asarray(segment_ids).astype(np.int64)

    in_maps, meta = _host_prep(bu, bi, Q, X, Y, user, item, imp_items,
                               imp_ratings, segment_ids)
    nc = _build_graph(meta)

    if _sim:
        from concourse import bass_interp
        sim = bass_interp.CoreSim(nc)
        sim.assign_tensors(in_maps[0])
        sim.simulate()
        out0 = np.array(sim.tensor("out"))
        return sim, out0, in_maps, meta

    from concourse.bass_utils import run_bass_kernel_spmd
    res = run_bass_kernel_spmd(nc, in_maps, core_ids=list(range(N_CORES)),
                               trace=False)
    out = np.concatenate([res.results[m]["out"] for m in range(N_CORES)])
    return out.astype(np.float32)
